# revision 4
# baseline (speedup 1.0000x reference)
"""Fused multi-head attention (B=4, S=2048, D=1024, H=16, Dh=64, RoPE) on 8 NeuronCores.

Sharding: core = (batch b, head-group g) with b = core//2, g = core%2.
Each core computes its batch's 8 heads end-to-end (qkv proj, RoPE, attention,
out-proj partial with Wout row-slice); host sums the two partials per batch.

On-device layout is "transposed" (features on partitions, sequence on the free
dim) so no on-device transposes are needed:
  A: qkT/kT = wqk.T @ xT   (f on partitions)  +  v = xT.T @ wv (natural [s, f])
     RoPE applied by DVE straight out of PSUM (sign folded into the sin table).
  B: simT[j,i] = krT.T @ qrT  per head (K=64), exp on ACT with fused 1/8 scale
     (no max subtraction: |sim| is O(6) for these inputs, exp is safe in fp32).
  C: outT_aug = v_aug.T @ expT  with a ones column in v_aug producing the
     softmax denominator for free (M=65).
  normalize: DVE reciprocal + GPSIMD partition broadcast + DVE multiply.
  D: finalT = wout.T @ outT.
All matmuls run in float32r (full PE rate, ~1e-4 relative rounding).
"""
import sys

for p in ("/opt/trn_rl_repo",):
    if p not in sys.path:
        sys.path.insert(0, p)

import numpy as np

import concourse.bacc as bacc
import concourse.bass as bass
import concourse.tile as tile
from concourse import mybir
from concourse.bass_utils import run_bass_kernel_spmd

P = 128
S = 2048
D = 1024
NH = 8            # heads per core
DH = 64
SB = 512          # matmul free-dim block
NSB = S // SB     # 4 s-blocks
KD = D // P       # 8 contraction tiles over d
ST = S // P       # 16 s partition-tiles (keys)
FV = NH * DH      # 512 features for this head group
N_CORES = 8
SCALE = DH ** -0.5

f32 = mybir.dt.float32
f32r = mybir.dt.float32r


def build_program():
    nc = bacc.Bacc("TRN2", target_bir_lowering=False, debug=False,
                   enable_asserts=False, num_devices=N_CORES)

    xT = nc.dram_tensor("xT", [D, S], f32r, kind="ExternalInput").ap()
    wqk = nc.dram_tensor("wqk", [D, 2 * FV], f32r, kind="ExternalInput").ap()
    wv = nc.dram_tensor("wv", [D, FV], f32r, kind="ExternalInput").ap()
    wout = nc.dram_tensor("wout", [FV, D], f32r, kind="ExternalInput").ap()
    cosb = nc.dram_tensor("cosb", [P, S], f32, kind="ExternalInput").ap()
    sinb = nc.dram_tensor("sinb", [P, S], f32, kind="ExternalInput").ap()
    outT = nc.dram_tensor("outT", [D, S], f32, kind="ExternalOutput").ap()

    with tile.TileContext(nc) as tc:
        with tc.tile_pool(name="persist", bufs=1) as pp, \
             tc.tile_pool(name="dram", bufs=1, space="DRAM") as dp:
            v_sb = [pp.tile([P, NH * (DH + 1)], f32r, tag=f"v{i}", name=f"v{i}") for i in range(ST)]
            outT_sb = [pp.tile([P, S], f32r, tag=f"ot{t}", name=f"ot{t}") for t in range(NSB)]
            qr_d = dp.tile([FV, S], f32r, tag="qr_d", name="qr_d")
            kr_d = dp.tile([FV, S], f32r, tag="kr_d", name="kr_d")

            # ones columns of v_aug
            ones8 = pp.tile([P, NH], f32, tag="ones8", name="ones8")
            nc.vector.memset(ones8[:], 1.0)
            for i in range(ST):
                ones_dst = v_sb[i].rearrange("p (h e) -> p h e", h=NH)[:, :, DH]
                nc.vector.tensor_copy(ones_dst, ones8[:])

            # ---------------- Phase A: projections + RoPE ----------------
            with tc.tile_pool(name="aw", bufs=1) as aw, \
                 tc.tile_pool(name="xtp", bufs=2) as xtp, \
                 tc.tile_pool(name="ropew", bufs=3) as rw, \
                 tc.tile_pool(name="stg", bufs=4) as stg, \
                 tc.tile_pool(name="psA", bufs=2, space="PSUM") as psA:
                wqk_sb = [aw.tile([P, 2 * FV], f32r, tag=f"wqk{k}", name=f"wqk{k}") for k in range(KD)]
                wv_sb = [aw.tile([P, FV], f32r, tag=f"wv{k}", name=f"wv{k}") for k in range(KD)]
                cos_sb = aw.tile([P, S], f32, tag="cos")
                sin_sb = aw.tile([P, S], f32, tag="sin")
                for k in range(KD):
                    nc.sync.dma_start(wqk_sb[k][:], wqk[P * k:P * (k + 1), :])
                    nc.sync.dma_start(wv_sb[k][:], wv[P * k:P * (k + 1), :])
                nc.sync.dma_start(cos_sb[:], cosb[:])
                nc.sync.dma_start(sin_sb[:], sinb[:])

                for nb in range(NSB):
                    sl = slice(nb * SB, (nb + 1) * SB)
                    xts = [xtp.tile([P, SB], f32r, tag=f"xt{k}", name=f"xt{k}") for k in range(KD)]
                    for k in range(KD):
                        nc.sync.dma_start(xts[k][:], xT[P * k:P * (k + 1), sl])

                    # q (m=0..3) and k (m=4..7) feature tiles, transposed layout
                    for m in range(2 * NSB):
                        ps = psA.tile([P, SB], f32, tag="psQK")
                        for k in range(KD):
                            nc.tensor.matmul(ps[:], wqk_sb[k][:, P * m:P * (m + 1)],
                                             xts[k][:], start=(k == 0), stop=(k == KD - 1))
                        # RoPE: out = q*cos + swap32(q)*sin_signed
                        qc = rw.tile([P, SB], f32, tag="qc")
                        tmp = rw.tile([P, SB], f32, tag="tmp")
                        nc.vector.tensor_mul(qc[:], ps[:], cos_sb[:, sl])
                        for blk in range(4):
                            a = 32 * blk
                            src = 32 * (blk ^ 1)
                            nc.vector.tensor_mul(tmp[a:a + 32, :],
                                                 ps[src:src + 32, :],
                                                 sin_sb[a:a + 32, sl])
                        dst = stg.tile([P, SB], f32r, tag="stg")
                        nc.vector.tensor_add(dst[:], qc[:], tmp[:])
                        tgt = qr_d if m < NSB else kr_d
                        mm = m % NSB
                        nc.sync.dma_start(tgt[P * mm:P * (mm + 1), sl], dst[:])

                    # v tiles, natural layout [s, f]
                    for st in range(NSB):
                        s_idx = nb * NSB + st
                        psv = psA.tile([P, FV], f32, tag="psV")
                        for k in range(KD):
                            nc.tensor.matmul(psv[:], xts[k][:, P * st:P * (st + 1)],
                                             wv_sb[k][:], start=(k == 0), stop=(k == KD - 1))
                        for h in range(NH):
                            nc.vector.tensor_copy(
                                v_sb[s_idx][:, (DH + 1) * h:(DH + 1) * h + DH],
                                psv[:, DH * h:DH * (h + 1)])

            # ---------------- Phase B/C/D: attention + out-proj ----------------
            with tc.tile_pool(name="bw", bufs=1) as bw, \
                 tc.tile_pool(name="qks", bufs=2) as qks, \
                 tc.tile_pool(name="expp", bufs=3) as expp, \
                 tc.tile_pool(name="nump", bufs=8) as nump, \
                 tc.tile_pool(name="bcp", bufs=4) as bcp, \
                 tc.tile_pool(name="rrp", bufs=2) as rrp, \
                 tc.tile_pool(name="doutp", bufs=3) as doutp, \
                 tc.tile_pool(name="psB", bufs=1, space="PSUM") as psB:
                wout_sb = [bw.tile([P, D], f32r, tag=f"wo{k}", name=f"wo{k}") for k in range(FV // P)]
                for k in range(FV // P):
                    nc.sync.dma_start(wout_sb[k][:], wout[P * k:P * (k + 1), :])

                for t in range(NSB):          # head pairs
                    qs = qks.tile([P, S], f32r, tag="qs")
                    ks = qks.tile([P, S], f32r, tag="ks")
                    nc.sync.dma_start(qs[:], qr_d[P * t:P * (t + 1), :])
                    nc.sync.dma_start(ks[:], kr_d[P * t:P * (t + 1), :])
                    for hh in range(2):
                        h = 2 * t + hh
                        off = DH * hh
                        rrow = rrp.tile([1, S], f32, tag="rrow")
                        for ip in range(2):   # i-block pairs
                            augs = [psB.tile([DH + 1, SB], f32, tag=f"aug{ii}", name=f"aug{ii}")
                                    for ii in range(2)]
                            for j in range(ST):
                                sim = psB.tile([P, 2 * SB], f32, tag="sim")
                                for ii in range(2):
                                    isl = slice((2 * ip + ii) * SB, (2 * ip + ii + 1) * SB)
                                    nc.tensor.matmul(sim[:, SB * ii:SB * (ii + 1)],
                                                     ks[off:off + DH, P * j:P * (j + 1)],
                                                     qs[off:off + DH, isl],
                                                     start=True, stop=True)
                                et = expp.tile([P, 2 * SB], f32r, tag="exp")
                                nc.scalar.activation(et[:], sim[:],
                                                     mybir.ActivationFunctionType.Exp,
                                                     scale=SCALE)
                                for ii in range(2):
                                    nc.tensor.matmul(augs[ii][:],
                                                     v_sb[j][:, (DH + 1) * h:(DH + 1) * h + DH + 1],
                                                     et[:, SB * ii:SB * (ii + 1)],
                                                     start=(j == 0), stop=(j == ST - 1))
                            for ii in range(2):
                                i_blk = 2 * ip + ii
                                isl = slice(i_blk * SB, (i_blk + 1) * SB)
                                num = nump.tile([DH, SB], f32, tag="num")
                                nc.vector.tensor_copy(num[:], augs[ii][0:DH, :])
                                nc.vector.reciprocal(rrow[0:1, isl], augs[ii][DH:DH + 1, :])
                                bc = bcp.tile([DH, SB], f32, tag="bc")
                                nc.gpsimd.partition_broadcast(bc[:], rrow[0:1, isl])
                                nc.vector.tensor_mul(outT_sb[t][off:off + DH, isl],
                                                     num[:], bc[:])

                # Phase D: final projection
                with tc.tile_pool(name="psD", bufs=2, space="PSUM") as psD:
                    for mi in range(D // P):
                        for ib in range(NSB):
                            isl = slice(ib * SB, (ib + 1) * SB)
                            pd = psD.tile([P, SB], f32, tag="pd")
                            for k in range(FV // P):
                                nc.tensor.matmul(pd[:], wout_sb[k][:, P * mi:P * (mi + 1)],
                                                 outT_sb[k][:, isl],
                                                 start=(k == 0), stop=(k == FV // P - 1))
                            ot = doutp.tile([P, SB], f32, tag="dout")
                            nc.vector.tensor_copy(ot[:], pd[:])
                            nc.sync.dma_start(outT[P * mi:P * (mi + 1), isl], ot[:])

    nc.compile()
    return nc


_PROG = None


def _get_prog():
    global _PROG
    if _PROG is None:
        _PROG = build_program()
    return _PROG


def make_in_maps(x, Wqkv, Wout):
    B = x.shape[0]
    HEADS = 16
    BASE = 10000.0
    # RoPE tables, sign folded into sin, 32-row frequency pattern tiled to 128
    f = np.arange(32, dtype=np.float64)
    invfreq = BASE ** (-2.0 * f / DH)                      # [32]
    tpos = np.arange(S, dtype=np.float64)
    ang = np.outer(invfreq, tpos)                          # [32, S]
    cos32 = np.cos(ang)
    sin32 = np.sin(ang)
    cosb = np.tile(cos32, (4, 1)).astype(np.float32)       # [128, S]
    sgn = np.repeat(np.array([-1.0, 1.0, -1.0, 1.0]), 32)[:, None]
    sinb = (np.tile(sin32, (4, 1)) * sgn).astype(np.float32)

    in_maps = []
    for c in range(N_CORES):
        b, g = divmod(c, 2)
        xTc = np.ascontiguousarray(x[b].T)                 # [D, S]
        wqk_c = np.ascontiguousarray(
            np.concatenate([Wqkv[:, 512 * g:512 * g + 512],
                            Wqkv[:, 1024 + 512 * g:1024 + 512 * g + 512]], axis=1))
        wv_c = np.ascontiguousarray(Wqkv[:, 2048 + 512 * g:2048 + 512 * g + 512])
        wout_c = np.ascontiguousarray(Wout[512 * g:512 * g + 512, :])
        in_maps.append({"xT": xTc, "wqk": wqk_c, "wv": wv_c, "wout": wout_c,
                        "cosb": cosb, "sinb": sinb})
    return in_maps


def gather_output(results, B=4):
    outs = []
    for b in range(B):
        acc = results[2 * b]["outT"].astype(np.float32) + results[2 * b + 1]["outT"]
        outs.append(acc.T)
    return np.stack(outs, axis=0)


def kernel(x, Wqkv, Wout):
    x = np.asarray(x, dtype=np.float32)
    Wqkv = np.asarray(Wqkv, dtype=np.float32)
    Wout = np.asarray(Wout, dtype=np.float32)
    nc = _get_prog()
    in_maps = make_in_maps(x, Wqkv, Wout)
    res = run_bass_kernel_spmd(nc, in_maps, core_ids=list(range(N_CORES)))
    return gather_output(res.results, B=x.shape[0])


if __name__ == "__main__":
    rng = np.random.default_rng(0)
    x = rng.standard_normal((4, S, D)).astype(np.float32)
    Wqkv = (rng.standard_normal((D, 3 * D)) * D ** -0.5).astype(np.float32)
    Wout = (rng.standard_normal((D, D)) * D ** -0.5).astype(np.float32)
    out = kernel(x, Wqkv, Wout)
    print("kernel ran, out shape:", out.shape, "finite:", np.isfinite(out).all())


# revision 6
# speedup vs baseline: 1.5849x; 1.5849x over previous
"""Fused multi-head attention (B=4, S=2048, D=1024, H=16, Dh=64, RoPE) on 8 NeuronCores.

Sharding: core = (batch b, head-group g) with b = core//2, g = core%2.
Each core computes its batch's 8 heads end-to-end (qkv proj, RoPE, attention,
out-proj partial with Wout row-slice); host sums the two partials per batch.

On-device layout is "transposed" (features on partitions, sequence on the free
dim) so no on-device transposes are needed:
  A: qkT/kT = wqk.T @ xT   (f on partitions)  +  v = xT.T @ wv (natural [s, f])
     RoPE applied by DVE straight out of PSUM (sign folded into the sin table).
  B: simT[j,i] = krT.T @ qrT  per head (K=64), exp on ACT with fused 1/8 scale
     (no max subtraction: |sim| is O(6) for these inputs, exp is safe in fp32).
  C: outT_aug = v_aug.T @ expT  with a ones column in v_aug producing the
     softmax denominator for free (M=65).
  normalize: DVE reciprocal + GPSIMD partition broadcast + DVE multiply.
  D: finalT = wout.T @ outT.
All matmuls run in float32r (full PE rate, ~1e-4 relative rounding).
"""
import sys

for p in ("/opt/trn_rl_repo",):
    if p not in sys.path:
        sys.path.insert(0, p)

import numpy as np

import concourse.bacc as bacc
import concourse.bass as bass
import concourse.tile as tile
from concourse import mybir
from concourse.bass_utils import run_bass_kernel_spmd

P = 128
S = 2048
D = 1024
NH = 8            # heads per core
DH = 64
SB = 512          # matmul free-dim block
NSB = S // SB     # 4 s-blocks
KD = D // P       # 8 contraction tiles over d
ST = S // P       # 16 s partition-tiles (keys)
FV = NH * DH      # 512 features for this head group
N_CORES = 8
SCALE = DH ** -0.5

f32 = mybir.dt.float32
f32r = mybir.dt.float32r


def build_program():
    nc = bacc.Bacc("TRN2", target_bir_lowering=False, debug=False,
                   enable_asserts=False, num_devices=N_CORES)

    xT = nc.dram_tensor("xT", [D, S], f32r, kind="ExternalInput").ap()
    wqk = nc.dram_tensor("wqk", [D, 2 * FV], f32r, kind="ExternalInput").ap()
    wv = nc.dram_tensor("wv", [D, FV], f32r, kind="ExternalInput").ap()
    wout = nc.dram_tensor("wout", [FV, D], f32r, kind="ExternalInput").ap()
    cosb = nc.dram_tensor("cosb", [P, S], f32, kind="ExternalInput").ap()
    sinb = nc.dram_tensor("sinb", [P, S], f32, kind="ExternalInput").ap()
    outT = nc.dram_tensor("outT", [D, S], f32, kind="ExternalOutput").ap()

    with tile.TileContext(nc) as tc:
        with tc.tile_pool(name="persist", bufs=1) as pp, \
             tc.tile_pool(name="dram", bufs=1, space="DRAM") as dp:
            v_sb = [pp.tile([P, NH * (DH + 1)], f32r, tag=f"v{i}", name=f"v{i}") for i in range(ST)]
            outT_sb = [pp.tile([P, S], f32r, tag=f"ot{t}", name=f"ot{t}") for t in range(NSB)]
            qr_d = dp.tile([FV, S], f32r, tag="qr_d", name="qr_d")
            kr_d = dp.tile([FV, S], f32r, tag="kr_d", name="kr_d")

            # ones columns of v_aug
            ones8 = pp.tile([P, NH], f32, tag="ones8", name="ones8")
            nc.vector.memset(ones8[:], 1.0)
            for i in range(ST):
                ones_dst = v_sb[i].rearrange("p (h e) -> p h e", h=NH)[:, :, DH]
                nc.vector.tensor_copy(ones_dst, ones8[:])

            # ---------------- Phase A: projections + RoPE ----------------
            with tc.tile_pool(name="aw", bufs=1) as aw, \
                 tc.tile_pool(name="xtp", bufs=2) as xtp, \
                 tc.tile_pool(name="ropew", bufs=3) as rw, \
                 tc.tile_pool(name="stg", bufs=4) as stg, \
                 tc.tile_pool(name="psA", bufs=3, space="PSUM") as psA:
                wqk_sb = [aw.tile([P, 2 * FV], f32r, tag=f"wqk{k}", name=f"wqk{k}") for k in range(KD)]
                wv_sb = [aw.tile([P, FV], f32r, tag=f"wv{k}", name=f"wv{k}") for k in range(KD)]
                cos_sb = aw.tile([P, S], f32, tag="cos")
                sin_sb = aw.tile([P, S], f32, tag="sin")
                for k in range(KD):
                    nc.sync.dma_start(wqk_sb[k][:], wqk[P * k:P * (k + 1), :])
                    nc.sync.dma_start(wv_sb[k][:], wv[P * k:P * (k + 1), :])
                nc.sync.dma_start(cos_sb[:], cosb[:])
                nc.sync.dma_start(sin_sb[:], sinb[:])

                for nb in range(NSB):
                    sl = slice(nb * SB, (nb + 1) * SB)
                    xts = [xtp.tile([P, SB], f32r, tag=f"xt{k}", name=f"xt{k}") for k in range(KD)]
                    for k in range(KD):
                        nc.sync.dma_start(xts[k][:], xT[P * k:P * (k + 1), sl])

                    # q (m=0..3) and k (m=4..7) feature tiles, transposed layout
                    for m in range(2 * NSB):
                        ps = psA.tile([P, SB], f32, tag="psQK")
                        for k in range(KD):
                            nc.tensor.matmul(ps[:], wqk_sb[k][:, P * m:P * (m + 1)],
                                             xts[k][:], start=(k == 0), stop=(k == KD - 1))
                        # RoPE: out = q*cos + swap32(q)*sin_signed
                        qc = rw.tile([P, SB], f32, tag="qc")
                        tmp = rw.tile([P, SB], f32, tag="tmp")
                        nc.vector.tensor_mul(qc[:], ps[:], cos_sb[:, sl])
                        for blk in range(4):
                            a = 32 * blk
                            src = 32 * (blk ^ 1)
                            nc.vector.tensor_mul(tmp[a:a + 32, :],
                                                 ps[src:src + 32, :],
                                                 sin_sb[a:a + 32, sl])
                        dst = stg.tile([P, SB], f32r, tag="stg")
                        nc.vector.tensor_add(dst[:], qc[:], tmp[:])
                        tgt = qr_d if m < NSB else kr_d
                        mm = m % NSB
                        nc.sync.dma_start(tgt[P * mm:P * (mm + 1), sl], dst[:])

                    # v tiles, natural layout [s, f]
                    for st in range(NSB):
                        s_idx = nb * NSB + st
                        psv = psA.tile([P, FV], f32, tag="psV")
                        for k in range(KD):
                            nc.tensor.matmul(psv[:], xts[k][:, P * st:P * (st + 1)],
                                             wv_sb[k][:], start=(k == 0), stop=(k == KD - 1))
                        for h in range(NH):
                            nc.vector.tensor_copy(
                                v_sb[s_idx][:, (DH + 1) * h:(DH + 1) * h + DH],
                                psv[:, DH * h:DH * (h + 1)])

            # ---------------- Phase B/C/D: attention + out-proj ----------------
            with tc.tile_pool(name="bw", bufs=1) as bw, \
                 tc.tile_pool(name="qks", bufs=2) as qks, \
                 tc.tile_pool(name="expp", bufs=4) as expp, \
                 tc.tile_pool(name="nump", bufs=8) as nump, \
                 tc.tile_pool(name="bcp", bufs=4) as bcp, \
                 tc.tile_pool(name="rrp", bufs=2) as rrp, \
                 tc.tile_pool(name="doutp", bufs=3) as doutp:
                wout_sb = [bw.tile([P, D], f32r, tag=f"wo{k}", name=f"wo{k}") for k in range(FV // P)]
                for k in range(FV // P):
                    nc.sync.dma_start(wout_sb[k][:], wout[P * k:P * (k + 1), :])

                with tc.tile_pool(name="psB", bufs=1, space="PSUM") as psB:
                    for t in range(NSB):          # head pairs
                        qs = qks.tile([P, S], f32r, tag="qs")
                        ks = qks.tile([P, S], f32r, tag="ks")
                        nc.sync.dma_start(qs[:], qr_d[P * t:P * (t + 1), :])
                        nc.sync.dma_start(ks[:], kr_d[P * t:P * (t + 1), :])
                        for hh in range(2):
                            h = 2 * t + hh
                            off = DH * hh
                            rrow = rrp.tile([1, S], f32, tag="rrow")
                            for ip in range(2):   # i-block pairs
                                augs = [psB.tile([DH + 1, SB], f32, tag=f"aug{ii}", name=f"aug{ii}")
                                        for ii in range(2)]

                                # software pipeline: emit C(j-2) after B(j)+exp(j)
                                # so the PE never waits on ACT latency
                                ets = {}

                                def emit_b(j):
                                    sim = psB.tile([P, 2 * SB], f32, tag="sim",
                                                   bufs=3, name="sim")
                                    for ii in range(2):
                                        isl = slice((2 * ip + ii) * SB, (2 * ip + ii + 1) * SB)
                                        nc.tensor.matmul(sim[:, SB * ii:SB * (ii + 1)],
                                                         ks[off:off + DH, P * j:P * (j + 1)],
                                                         qs[off:off + DH, isl],
                                                         start=True, stop=True)
                                    et = expp.tile([P, 2 * SB], f32r, tag="exp", name="et")
                                    nc.scalar.activation(et[:], sim[:],
                                                         mybir.ActivationFunctionType.Exp,
                                                         scale=SCALE)
                                    ets[j] = et

                                def emit_c(j):
                                    et = ets.pop(j)
                                    for ii in range(2):
                                        nc.tensor.matmul(augs[ii][:],
                                                         v_sb[j][:, (DH + 1) * h:(DH + 1) * h + DH + 1],
                                                         et[:, SB * ii:SB * (ii + 1)],
                                                         start=(j == 0), stop=(j == ST - 1))

                                DEPTH = 2
                                for j in range(ST):
                                    emit_b(j)
                                    if j >= DEPTH:
                                        emit_c(j - DEPTH)
                                for j in range(ST - DEPTH, ST):
                                    emit_c(j)

                                for ii in range(2):
                                    i_blk = 2 * ip + ii
                                    isl = slice(i_blk * SB, (i_blk + 1) * SB)
                                    num = nump.tile([DH, SB], f32, tag="num")
                                    nc.vector.tensor_copy(num[:], augs[ii][0:DH, :])
                                    nc.vector.reciprocal(rrow[0:1, isl], augs[ii][DH:DH + 1, :])
                                    bc = bcp.tile([DH, SB], f32, tag="bc")
                                    nc.gpsimd.partition_broadcast(bc[:], rrow[0:1, isl])
                                    nc.vector.tensor_mul(outT_sb[t][off:off + DH, isl],
                                                         num[:], bc[:])

                # Phase D: final projection
                with tc.tile_pool(name="psD", bufs=2, space="PSUM") as psD:
                    for mi in range(D // P):
                        for ib in range(NSB):
                            isl = slice(ib * SB, (ib + 1) * SB)
                            pd = psD.tile([P, SB], f32, tag="pd")
                            for k in range(FV // P):
                                nc.tensor.matmul(pd[:], wout_sb[k][:, P * mi:P * (mi + 1)],
                                                 outT_sb[k][:, isl],
                                                 start=(k == 0), stop=(k == FV // P - 1))
                            ot = doutp.tile([P, SB], f32, tag="dout")
                            nc.vector.tensor_copy(ot[:], pd[:])
                            nc.sync.dma_start(outT[P * mi:P * (mi + 1), isl], ot[:])

    nc.compile()
    return nc


_PROG = None


def _get_prog():
    global _PROG
    if _PROG is None:
        _PROG = build_program()
    return _PROG


def make_in_maps(x, Wqkv, Wout):
    B = x.shape[0]
    HEADS = 16
    BASE = 10000.0
    # RoPE tables, sign folded into sin, 32-row frequency pattern tiled to 128
    f = np.arange(32, dtype=np.float64)
    invfreq = BASE ** (-2.0 * f / DH)                      # [32]
    tpos = np.arange(S, dtype=np.float64)
    ang = np.outer(invfreq, tpos)                          # [32, S]
    cos32 = np.cos(ang)
    sin32 = np.sin(ang)
    cosb = np.tile(cos32, (4, 1)).astype(np.float32)       # [128, S]
    sgn = np.repeat(np.array([-1.0, 1.0, -1.0, 1.0]), 32)[:, None]
    sinb = (np.tile(sin32, (4, 1)) * sgn).astype(np.float32)

    in_maps = []
    for c in range(N_CORES):
        b, g = divmod(c, 2)
        xTc = np.ascontiguousarray(x[b].T)                 # [D, S]
        wqk_c = np.ascontiguousarray(
            np.concatenate([Wqkv[:, 512 * g:512 * g + 512],
                            Wqkv[:, 1024 + 512 * g:1024 + 512 * g + 512]], axis=1))
        wv_c = np.ascontiguousarray(Wqkv[:, 2048 + 512 * g:2048 + 512 * g + 512])
        wout_c = np.ascontiguousarray(Wout[512 * g:512 * g + 512, :])
        in_maps.append({"xT": xTc, "wqk": wqk_c, "wv": wv_c, "wout": wout_c,
                        "cosb": cosb, "sinb": sinb})
    return in_maps


def gather_output(results, B=4):
    outs = []
    for b in range(B):
        acc = results[2 * b]["outT"].astype(np.float32) + results[2 * b + 1]["outT"]
        outs.append(acc.T)
    return np.stack(outs, axis=0)


def kernel(x, Wqkv, Wout):
    x = np.asarray(x, dtype=np.float32)
    Wqkv = np.asarray(Wqkv, dtype=np.float32)
    Wout = np.asarray(Wout, dtype=np.float32)
    nc = _get_prog()
    in_maps = make_in_maps(x, Wqkv, Wout)
    res = run_bass_kernel_spmd(nc, in_maps, core_ids=list(range(N_CORES)))
    return gather_output(res.results, B=x.shape[0])


if __name__ == "__main__":
    rng = np.random.default_rng(0)
    x = rng.standard_normal((4, S, D)).astype(np.float32)
    Wqkv = (rng.standard_normal((D, 3 * D)) * D ** -0.5).astype(np.float32)
    Wout = (rng.standard_normal((D, D)) * D ** -0.5).astype(np.float32)
    out = kernel(x, Wqkv, Wout)
    print("kernel ran, out shape:", out.shape, "finite:", np.isfinite(out).all())


# revision 12
# speedup vs baseline: 1.8080x; 1.1408x over previous
"""Fused multi-head attention (B=4, S=2048, D=1024, H=16, Dh=64, RoPE) on 8 NeuronCores.

Sharding: core = (batch b, head-group g) with b = core//2, g = core%2.
Each core computes its batch's 8 heads end-to-end (qkv proj, RoPE, attention,
out-proj partial with Wout row-slice); host sums the two partials per batch.

On-device layout is "transposed" (features on partitions, sequence on the free
dim) so no on-device transposes are needed:
  A: qkT/kT = wqk.T @ xT   (f on partitions)  +  v = xT.T @ wv (natural [s, f])
     RoPE applied by DVE straight out of PSUM (sign folded into the sin table).
  B: simT[j,i] = krT.T @ qrT  per head (K=64), exp on ACT with fused 1/8 scale
     (no max subtraction: |sim| is O(6) for these inputs, exp is safe in fp32).
  C: outT_aug = v_aug.T @ expT  with a ones column in v_aug producing the
     softmax denominator for free (M=65).
  normalize: DVE reciprocal + GPSIMD partition broadcast + DVE multiply.
  D: finalT = wout.T @ outT.
All matmuls run in float32r (full PE rate, ~1e-4 relative rounding).
"""
import sys

for p in ("/opt/trn_rl_repo",):
    if p not in sys.path:
        sys.path.insert(0, p)

import numpy as np

import concourse.bacc as bacc
import concourse.bass as bass
import concourse.tile as tile
from concourse import mybir
from concourse.bass_utils import run_bass_kernel_spmd

P = 128
S = 2048
D = 1024
NH = 8            # heads per core
DH = 64
SB = 512          # matmul free-dim block
NSB = S // SB     # 4 s-blocks
KD = D // P       # 8 contraction tiles over d
ST = S // P       # 16 s partition-tiles (keys)
FV = NH * DH      # 512 features for this head group
N_CORES = 8
SCALE = DH ** -0.5

f32 = mybir.dt.float32
f32r = mybir.dt.float32r
bf16 = mybir.dt.bfloat16


def build_program():
    nc = bacc.Bacc("TRN2", target_bir_lowering=False, debug=False,
                   enable_asserts=False, num_devices=N_CORES)

    xT = nc.dram_tensor("xT", [D, S], f32r, kind="ExternalInput").ap()
    wqk = nc.dram_tensor("wqk", [D, 2 * FV], f32r, kind="ExternalInput").ap()
    wv = nc.dram_tensor("wv", [D, FV], f32r, kind="ExternalInput").ap()
    wout = nc.dram_tensor("wout", [FV, D], f32r, kind="ExternalInput").ap()
    cosb = nc.dram_tensor("cosb", [P, S], f32, kind="ExternalInput").ap()
    sinb = nc.dram_tensor("sinb", [P, S], f32, kind="ExternalInput").ap()
    outT = nc.dram_tensor("outT", [D, S], f32, kind="ExternalOutput").ap()

    with tile.TileContext(nc) as tc:
        with tc.tile_pool(name="persist", bufs=1) as pp, \
             tc.tile_pool(name="dram", bufs=1, space="DRAM") as dp:
            v_sb = [pp.tile([P, NH * (DH + 1)], f32r, tag=f"v{i}", name=f"v{i}") for i in range(ST)]
            outT_sb = [pp.tile([P, S], f32r, tag=f"ot{t}", name=f"ot{t}") for t in range(NSB)]
            # cos/sin partial products round-trip through DRAM in bf16; the
            # rotate-half partition swap happens in the read-back addressing.
            qc_d = dp.tile([FV, S], bf16, tag="qc_d", name="qc_d")
            kc_d = dp.tile([FV, S], bf16, tag="kc_d", name="kc_d")
            qs_d = dp.tile([FV, S], bf16, tag="qs_d", name="qs_d")
            ks_d = dp.tile([FV, S], bf16, tag="ks_d", name="ks_d")

            # ones columns of v_aug
            ones8 = pp.tile([P, NH], f32, tag="ones8", name="ones8")
            nc.vector.memset(ones8[:], 1.0)
            for i in range(ST):
                ones_dst = v_sb[i].rearrange("p (h e) -> p h e", h=NH)[:, :, DH]
                nc.vector.tensor_copy(ones_dst, ones8[:])

            # ---------------- Phase A: projections + RoPE ----------------
            with tc.tile_pool(name="aw", bufs=1) as aw, \
                 tc.tile_pool(name="xtp", bufs=2) as xtp, \
                 tc.tile_pool(name="ropew", bufs=3) as rw, \
                 tc.tile_pool(name="psA", bufs=3, space="PSUM") as psA:
                wqk_sb = [aw.tile([P, 2 * FV], f32r, tag=f"wqk{k}", name=f"wqk{k}") for k in range(KD)]
                wv_sb = [aw.tile([P, FV], f32r, tag=f"wv{k}", name=f"wv{k}") for k in range(KD)]
                cos_sb = aw.tile([P, S], f32, tag="cos")
                sin_sb = aw.tile([P, S], f32, tag="sin")
                for k in range(KD):
                    nc.sync.dma_start(wqk_sb[k][:], wqk[P * k:P * (k + 1), :])
                    nc.sync.dma_start(wv_sb[k][:], wv[P * k:P * (k + 1), :])
                nc.sync.dma_start(cos_sb[:], cosb[:])
                nc.sync.dma_start(sin_sb[:], sinb[:])

                for nb in range(NSB):
                    sl = slice(nb * SB, (nb + 1) * SB)
                    xts = [xtp.tile([P, SB], f32r, tag=f"xt{k}", name=f"xt{k}") for k in range(KD)]
                    for k in range(KD):
                        nc.sync.dma_start(xts[k][:], xT[P * k:P * (k + 1), sl])

                    # q (m=0..3) and k (m=4..7) feature tiles, transposed layout
                    for m in range(2 * NSB):
                        ps = psA.tile([P, SB], f32, tag="psQK")
                        for k in range(KD):
                            nc.tensor.matmul(ps[:], wqk_sb[k][:, P * m:P * (m + 1)],
                                             xts[k][:], start=(k == 0), stop=(k == KD - 1))
                        # RoPE partial products (swap + add happen in phase B)
                        qc = rw.tile([P, SB], bf16, tag="qc")
                        qsn = rw.tile([P, SB], bf16, tag="qsn")
                        nc.vector.tensor_mul(qc[:], ps[:], cos_sb[:, sl])
                        nc.vector.tensor_mul(qsn[:], ps[:], sin_sb[:, sl])
                        ctgt = qc_d if m < NSB else kc_d
                        stgt = qs_d if m < NSB else ks_d
                        mm = m % NSB
                        nc.sync.dma_start(ctgt[P * mm:P * (mm + 1), sl], qc[:])
                        nc.sync.dma_start(stgt[P * mm:P * (mm + 1), sl], qsn[:])

                    # v tiles, natural layout [s, f]
                    for st in range(NSB):
                        s_idx = nb * NSB + st
                        psv = psA.tile([P, FV], f32, tag="psV")
                        for k in range(KD):
                            nc.tensor.matmul(psv[:], xts[k][:, P * st:P * (st + 1)],
                                             wv_sb[k][:], start=(k == 0), stop=(k == KD - 1))
                        for h in range(NH):
                            nc.vector.tensor_copy(
                                v_sb[s_idx][:, (DH + 1) * h:(DH + 1) * h + DH],
                                psv[:, DH * h:DH * (h + 1)])

            # ---------------- Phase B/C/D: attention + out-proj ----------------
            with tc.tile_pool(name="bw", bufs=1) as bw, \
                 tc.tile_pool(name="qks", bufs=2) as qks, \
                 tc.tile_pool(name="expp", bufs=4) as expp, \
                 tc.tile_pool(name="nump", bufs=8) as nump, \
                 tc.tile_pool(name="bcp", bufs=4) as bcp, \
                 tc.tile_pool(name="rrp", bufs=2) as rrp, \
                 tc.tile_pool(name="doutp", bufs=3) as doutp:
                wout_sb = [bw.tile([P, D], f32r, tag=f"wo{k}", name=f"wo{k}") for k in range(FV // P)]
                for k in range(FV // P):
                    nc.sync.dma_start(wout_sb[k][:], wout[P * k:P * (k + 1), :])

                def load_roped(t, c_d, s_d, tagpfx):
                    """Load cos/sin partials for pair t; sin partials read with
                    the rotate-half partition swap; add into an f32r tile."""
                    ct = qks.tile([P, S], bf16, tag=f"{tagpfx}c", name="ct")
                    sw = qks.tile([P, S], bf16, tag=f"{tagpfx}w", name="sw")
                    nc.sync.dma_start(ct[:], c_d[P * t:P * (t + 1), :])
                    for blk in range(4):
                        a = 32 * blk
                        src = P * t + 32 * (blk ^ 1)
                        nc.sync.dma_start(sw[a:a + 32, :], s_d[src:src + 32, :])
                    r = qks.tile([P, S], f32r, tag=f"{tagpfx}r", name="r")
                    nc.vector.tensor_add(r[:], ct[:], sw[:])
                    return r

                with tc.tile_pool(name="psB", bufs=1, space="PSUM") as psB:
                    for t in range(NSB):          # head pairs
                        qs = load_roped(t, qc_d, qs_d, "q")
                        ks = load_roped(t, kc_d, ks_d, "k")
                        for hh in range(2):
                            h = 2 * t + hh
                            off = DH * hh
                            rrow = rrp.tile([1, S], f32, tag="rrow")
                            for ip in range(2):   # i-block pairs
                                augs = [psB.tile([DH + 1, SB], f32, tag=f"aug{ii}", name=f"aug{ii}")
                                        for ii in range(2)]

                                # software pipeline: emit C(j-2) after B(j)+exp(j)
                                # so the PE never waits on ACT latency
                                ets = {}

                                def emit_b(j):
                                    sim = psB.tile([P, 2 * SB], f32, tag="sim",
                                                   bufs=3, name="sim")
                                    for ii in range(2):
                                        isl = slice((2 * ip + ii) * SB, (2 * ip + ii + 1) * SB)
                                        nc.tensor.matmul(sim[:, SB * ii:SB * (ii + 1)],
                                                         ks[off:off + DH, P * j:P * (j + 1)],
                                                         qs[off:off + DH, isl],
                                                         start=True, stop=True)
                                    et = expp.tile([P, 2 * SB], f32r, tag="exp", name="et")
                                    nc.scalar.activation(et[:], sim[:],
                                                         mybir.ActivationFunctionType.Exp,
                                                         scale=SCALE)
                                    ets[j] = et

                                def emit_c(j):
                                    et = ets.pop(j)
                                    for ii in range(2):
                                        nc.tensor.matmul(augs[ii][:],
                                                         v_sb[j][:, (DH + 1) * h:(DH + 1) * h + DH + 1],
                                                         et[:, SB * ii:SB * (ii + 1)],
                                                         start=(j == 0), stop=(j == ST - 1))

                                DEPTH = 2
                                for j in range(ST):
                                    emit_b(j)
                                    if j >= DEPTH:
                                        emit_c(j - DEPTH)
                                for j in range(ST - DEPTH, ST):
                                    emit_c(j)

                                for ii in range(2):
                                    i_blk = 2 * ip + ii
                                    isl = slice(i_blk * SB, (i_blk + 1) * SB)
                                    num = nump.tile([DH, SB], f32, tag="num")
                                    nc.vector.tensor_copy(num[:], augs[ii][0:DH, :])
                                    nc.vector.reciprocal(rrow[0:1, isl], augs[ii][DH:DH + 1, :])
                                    bc = bcp.tile([DH, SB], f32, tag="bc")
                                    nc.gpsimd.partition_broadcast(bc[:], rrow[0:1, isl])
                                    nc.vector.tensor_mul(outT_sb[t][off:off + DH, isl],
                                                         num[:], bc[:])

                # Phase D: final projection
                with tc.tile_pool(name="psD", bufs=2, space="PSUM") as psD:
                    for mi in range(D // P):
                        for ib in range(NSB):
                            isl = slice(ib * SB, (ib + 1) * SB)
                            pd = psD.tile([P, SB], f32, tag="pd")
                            for k in range(FV // P):
                                nc.tensor.matmul(pd[:], wout_sb[k][:, P * mi:P * (mi + 1)],
                                                 outT_sb[k][:, isl],
                                                 start=(k == 0), stop=(k == FV // P - 1))
                            ot = doutp.tile([P, SB], f32, tag="dout")
                            nc.vector.tensor_copy(ot[:], pd[:])
                            nc.sync.dma_start(outT[P * mi:P * (mi + 1), isl], ot[:])

    nc.compile()
    return nc


_PROG = None


def _get_prog():
    global _PROG
    if _PROG is None:
        _PROG = build_program()
    return _PROG


def make_in_maps(x, Wqkv, Wout):
    B = x.shape[0]
    HEADS = 16
    BASE = 10000.0
    # RoPE tables, sign folded into sin, 32-row frequency pattern tiled to 128
    f = np.arange(32, dtype=np.float64)
    invfreq = BASE ** (-2.0 * f / DH)                      # [32]
    tpos = np.arange(S, dtype=np.float64)
    ang = np.outer(invfreq, tpos)                          # [32, S]
    cos32 = np.cos(ang)
    sin32 = np.sin(ang)
    cosb = np.tile(cos32, (4, 1)).astype(np.float32)       # [128, S]
    # sign indexed by SOURCE row r: the swap moves row r to row swap(r), which
    # needs -sin when swap(r)%64 < 32, i.e. when r%64 >= 32
    sgn = np.repeat(np.array([1.0, -1.0, 1.0, -1.0]), 32)[:, None]
    sinb = (np.tile(sin32, (4, 1)) * sgn).astype(np.float32)

    in_maps = []
    for c in range(N_CORES):
        b, g = divmod(c, 2)
        xTc = np.ascontiguousarray(x[b].T)                 # [D, S]
        wqk_c = np.ascontiguousarray(
            np.concatenate([Wqkv[:, 512 * g:512 * g + 512],
                            Wqkv[:, 1024 + 512 * g:1024 + 512 * g + 512]], axis=1))
        wv_c = np.ascontiguousarray(Wqkv[:, 2048 + 512 * g:2048 + 512 * g + 512])
        wout_c = np.ascontiguousarray(Wout[512 * g:512 * g + 512, :])
        in_maps.append({"xT": xTc, "wqk": wqk_c, "wv": wv_c, "wout": wout_c,
                        "cosb": cosb, "sinb": sinb})
    return in_maps


def gather_output(results, B=4):
    outs = []
    for b in range(B):
        acc = results[2 * b]["outT"].astype(np.float32) + results[2 * b + 1]["outT"]
        outs.append(acc.T)
    return np.stack(outs, axis=0)


def kernel(x, Wqkv, Wout):
    x = np.asarray(x, dtype=np.float32)
    Wqkv = np.asarray(Wqkv, dtype=np.float32)
    Wout = np.asarray(Wout, dtype=np.float32)
    nc = _get_prog()
    in_maps = make_in_maps(x, Wqkv, Wout)
    res = run_bass_kernel_spmd(nc, in_maps, core_ids=list(range(N_CORES)))
    return gather_output(res.results, B=x.shape[0])


if __name__ == "__main__":
    rng = np.random.default_rng(0)
    x = rng.standard_normal((4, S, D)).astype(np.float32)
    Wqkv = (rng.standard_normal((D, 3 * D)) * D ** -0.5).astype(np.float32)
    Wout = (rng.standard_normal((D, D)) * D ** -0.5).astype(np.float32)
    out = kernel(x, Wqkv, Wout)
    print("kernel ran, out shape:", out.shape, "finite:", np.isfinite(out).all())
